# revision 11
# baseline (speedup 1.0000x reference)
"""Trainium2 Bass kernel: topo-batched masked-norm NN forward (gnn_message_passing).

Math per topo batch i (reference.py):
    vals = previous layer activations [W]
    n_in[r]  = sum_c M[r,c]                       (host-precomputed -> rn = 1/n_in)
    mean[r]  = (M @ vals)[r] / n_in[r]
    var[r]   = (M @ vals^2)[r] / n_in[r] - mean[r]^2
    rs[r]    = 1/sqrt(var[r] + EPS)
    affine[r]= gamma*rs*(WM @ vals)[r] + (beta - gamma*rs*mean)[r]*(WM @ 1)[r] + bias[r]
        where WM = W (.) M   (algebraic expansion of the masked-norm + masked affine)
    out = silu(affine*gain)*amp   (last batch: identity instead of silu)

Distribution: rows (output neurons) sharded across 8 cores (512 rows/core);
the 4096-vector of activations is all-gathered between batches.

v2 design (from trace analysis of the hi/lo baseline at 529us):
  - ONLY WM is shipped, as single bf16 (weights were f32-split before; the
    2e-2 tolerance leaves ample room). The 0/1 mask is derived ON DEVICE as
    M = (WM != 0) -- one DVE pass -- since WM is nonzero exactly where the
    mask is set (host clamps the impossible collisions). HBM traffic drops
    2.5x and the 16 per-batch DVE mask-multiplies disappear.
  - The matvec sweep runs stats (M@*, 5 stationary cols) and affine (WM@*,
    3 cols) CONCURRENTLY in different PE column groups via tile_position
    (0,0)/(0,32): the two 512-col streams overlap, halving sweep time.
  - Bulk weight DMA rides the scalar (ACT) HWDGE ring; the collective
    payload + vals DMAs ride the sync (SP) ring so the AllGather trigger
    no longer queues behind megabyte weight transfers.
  - The all-gather payload is consumed through a host-side column
    permutation so each partition reads one contiguous 128B run instead of
    eight 16B strides.
  - Epilogue: 1/n_in from the host, rsqrt = DVE reciprocal + ACT Sqrt,
    silu on ACT -- ~15 short DVE ops instead of ~25.
"""

import numpy as np
import ml_dtypes

import concourse.bass as bass
import concourse.bacc as bacc
import concourse.tile as tile
import concourse.mybir as mybir
from concourse import bass_utils

L, W, NC = 8, 4096, 8
NB = L - 1                # 7 topo batches
RPC = W // NC             # 512 rows per core
CB = W // 128             # 32 contraction blocks of 128
RB = RPC // 128           # 4 row blocks of 128 per core
HJ = CB // 2              # c-blocks per half tile (16)
EPS = 1e-5

BF16 = mybir.dt.bfloat16
F32 = mybir.dt.float32
I32 = mybir.dt.int32
ADD = mybir.AluOpType.add
SUB = mybir.AluOpType.subtract
MUL = mybir.AluOpType.mult
NE = mybir.AluOpType.not_equal
RSHIFT = mybir.AluOpType.logical_shift_right
ACTF = mybir.ActivationFunctionType

_CACHED = None


def _kernel_body(nc, tc, wm_d, xf_d, pf_d, sel_d, y_d):
    NP = 6  # per-row params: gamma, beta, bias, gain, amp, rn
    NQ = 4                    # weight DMA chunks per batch (8 c-blocks each)
    QJ = CB // NQ
    with (
        tc.tile_pool(name="const", bufs=1) as constp,
        tc.tile_pool(name="wm", bufs=2 * NQ) as wmp,
        tc.tile_pool(name="mk", bufs=2 * NQ) as mkp,
        tc.tile_pool(name="vals", bufs=2) as valsp,
        tc.tile_pool(name="ep", bufs=2) as epp,
        tc.tile_pool(name="sb8", bufs=1) as sb8p,
        tc.tile_pool(name="psum", bufs=1, space="PSUM") as psump,
        tc.tile_pool(name="dram", bufs=2, space="DRAM") as dramp,
    ):
        # ---- persistent: per-row params, folded [128, NB*NP*RB] ----
        params = constp.tile([128, NB * NP * RB], F32)
        nc.sync.dma_start(out=params[:], in_=pf_d.ap())

        def pslice(i, s):
            o = (i * NP + s) * RB
            return params[:, o:o + RB]

        # combined row-selector for the PSUM fold transpose (rows 0-4 stats,
        # rows 32-34 affine; rows 5-31 are zero)
        sel = constp.tile([128, 8], F32, name="sel")
        nc.sync.dma_start(out=sel[0:35, :], in_=sel_d.ap())

        # ---- persistent: per-batch stationary vectors [128, CB*5] bf16 ----
        # col layout per c-block j: [vhi, vlo, ones, sqhi, sqlo]
        vstat = constp.tile([128, CB * 5], BF16)
        v3 = vstat[:].rearrange("p (j s) -> p j s", s=5)
        nc.vector.memset(v3[:, :, 2], 1.0)

        # single SBUF staging tile for the fold transpose; partitions 5-31
        # feed zero selector rows but are still streamed through the PE, so
        # they must hold real numbers, not stale-SBUF bit patterns
        sb = sb8p.tile([128, 512], F32, tag="sb", name="sb")
        nc.vector.memset(sb[0:32, :], 0.0)

        # scratch for the keep-warm ladder: gpsimd self-clocks 5 slow ops
        # through the all-gather window; a tiny matmul after each one keeps
        # the PE HAM from re-throttling to 1.2 GHz between sweeps
        NWARM = 5
        warm_sb = constp.tile([128, NWARM * 256], F32, name="warm_sb")
        nc.vector.memset(warm_sb[:], 0.0)
        ps_w = None

        # ---- weight streaming + on-device mask (prefetched one batch ahead:
        # the DMA lands and the IS_NE mask-derive runs during the PREVIOUS
        # batch's sweep, keeping both off the epilogue's critical path).
        # The dma_starts are issued at high priority: at natural (body-order)
        # priority the list scheduler plans the dispatch after the previous
        # epilogue, which at runtime starves the sweep-window of weights ----
        def issue_weights(i):
            wm_t, m_t = [], []
            for q in range(NQ):
                wt = wmp.tile([128, QJ * RPC], BF16, tag="wm", name="wm")
                with tc.high_priority(offset=250):
                    nc.scalar.dma_start(
                        out=wt[:].rearrange("p (a b) -> p a b", b=RPC),
                        in_=wm_d[i][:, q * QJ:(q + 1) * QJ, :],
                    )
                mt = mkp.tile([128, QJ * RPC], BF16, tag="mk", name="mk")
                EW = QJ * RPC // 2
                for e in range(2):
                    nc.vector.tensor_scalar(
                        mt[:, e * EW:(e + 1) * EW],
                        wt[:, e * EW:(e + 1) * EW],
                        0.0, None, op0=NE)
                wm_t.append(wt)
                m_t.append(mt)
            return wm_t, m_t

        cur_w = issue_weights(0)
        prev_cc_out = None
        for i in range(NB):
            wm_t, m_t = cur_w

            # ============ vals -> vstat ============
            vals = valsp.tile([128, CB], F32, tag="vals", name="vals")
            if i == 0:
                nc.sync.dma_start(out=vals[:], in_=xf_d.ap())
            else:
                # payload idx = p*CB + j by construction (host permutes the
                # weight c-axis to match) => contiguous 128B per partition
                nc.sync.dma_start(
                    out=vals[:],
                    in_=prev_cc_out.rearrange("(p j) -> p j", j=CB),
                )
            tmp_a = epp.tile([128, CB], F32, tag="vtmp_a", name="vtmp_a")
            tmp_sq = epp.tile([128, CB], F32, tag="vtmp_sq", name="vtmp_sq")
            nc.vector.tensor_copy(v3[:, :, 0], vals[:])            # vhi
            nc.vector.tensor_copy(tmp_a[:], v3[:, :, 0])
            nc.vector.tensor_tensor(v3[:, :, 1], vals[:], tmp_a[:], op=SUB)
            nc.vector.tensor_tensor(tmp_sq[:], vals[:], vals[:], op=MUL)
            nc.vector.tensor_copy(v3[:, :, 3], tmp_sq[:])          # sqhi
            nc.vector.tensor_copy(tmp_a[:], v3[:, :, 3])
            nc.vector.tensor_tensor(v3[:, :, 4], tmp_sq[:], tmp_a[:], op=SUB)

            # prefetch next batch's weights + mask (runs under this sweep)
            if i + 1 < NB:
                cur_w = issue_weights(i + 1)

            # ============ matvec sweep (two concurrent column groups) ======
            # ps_st rows 0-4: [M@vhi, M@vlo, M@1, M@sqhi, M@sqlo]
            # ps_af rows 32-34: [WM@vhi, WM@vlo, WM@1]
            ps_st = psump.tile([128, 512], F32, tag="ps_st", name="ps_st")
            ps_af = psump.tile([128, 512], F32, tag="ps_af", name="ps_af")
            for j in range(CB):
                q, jq = divmod(j, QJ)
                rhs_m = m_t[q][:, jq * RPC:(jq + 1) * RPC]
                rhs_w = wm_t[q][:, jq * RPC:(jq + 1) * RPC]
                st, sp = (j == 0), (j == CB - 1)
                nc.tensor.matmul(ps_st[0:5, :], lhsT=vstat[:, j * 5:j * 5 + 5],
                                 rhs=rhs_m, start=st, stop=sp,
                                 tile_position=(0, 0))
                nc.tensor.matmul(ps_af[32:35, :], lhsT=vstat[:, j * 5:j * 5 + 3],
                                 rhs=rhs_w, start=st, stop=sp,
                                 tile_position=(0, 32))

            # ============ transpose to fold layout ============
            # sb rows: 0-4 = stats, 32-34 = affine; one 35-row selector
            # matmul per row block lands [128, 8] in PSUM with cols
            # [s1, n_in, s2, t1, rowWM] (bf16 hi/lo partials summed free)
            nc.vector.tensor_copy(sb[0:5, :], ps_st[0:5, :])
            nc.vector.tensor_copy(sb[32:35, :], ps_af[32:35, :])
            ps_t = psump.tile([128, RB * 512], F32, tag="ps_t", name="ps_t")
            for rb in range(RB):
                nc.tensor.matmul(
                    ps_t[:, rb * 512:rb * 512 + 8],
                    lhsT=sb[0:35, rb * 128:(rb + 1) * 128],
                    rhs=sel[0:35, :], start=True, stop=True)
            pt3 = ps_t[:].rearrange("p (rb s) -> p rb s", s=512)

            # ============ epilogue (all [128, RB] f32) ============
            def T(tag):
                return epp.tile([128, RB], F32, tag=tag, name=tag)

            # pt3 cols: 0=s1, 1=n_in(unused), 2=s2, 3=t1, 4=rowWM
            # params s: 0=gamma 1=beta 2=bias 3=gain 4=amp 5=rn
            mean, ex2, msq, vpe = T("mean"), T("ex2"), T("msq"), T("vpe")
            nc.vector.tensor_tensor(mean[:], pt3[:, :, 0], pslice(i, 5), op=MUL)
            nc.vector.tensor_tensor(ex2[:], pt3[:, :, 2], pslice(i, 5), op=MUL)
            nc.vector.tensor_tensor(msq[:], mean[:], mean[:], op=MUL)
            nc.vector.scalar_tensor_tensor(
                vpe[:], msq[:], -1.0, ex2[:], op0=MUL, op1=ADD)
            nc.vector.tensor_scalar(vpe[:], vpe[:], EPS, None, op0=ADD)
            # rs = 1/sqrt(vpe): Quake seed + 2 Newton iterations (DVE only --
            # an ACT Sqrt would thrash the activation table against Silu)
            rs = T("rs")
            nc.vector.tensor_scalar(
                rs[:].bitcast(I32), vpe[:].bitcast(I32), 1, None, op0=RSHIFT)
            nc.vector.tensor_scalar(
                rs[:].bitcast(I32), rs[:].bitcast(I32), -1, 0x5F3759DF,
                op0=MUL, op1=ADD)
            nra, nrb = T("nra"), T("nrb")
            for _ in range(2):
                nc.vector.tensor_tensor(nra[:], rs[:], rs[:], op=MUL)
                nc.vector.tensor_tensor(nrb[:], nra[:], vpe[:], op=MUL)
                nc.vector.tensor_scalar(nrb[:], nrb[:], -0.5, 1.5, op0=MUL, op1=ADD)
                nc.vector.tensor_tensor(rs[:], rs[:], nrb[:], op=MUL)
            g1, gm, coef = T("g1"), T("gm"), T("coef")
            nc.vector.tensor_tensor(g1[:], pslice(i, 0), rs[:], op=MUL)
            nc.vector.tensor_tensor(gm[:], g1[:], mean[:], op=MUL)
            nc.vector.tensor_tensor(coef[:], pslice(i, 1), gm[:], op=SUB)
            te1, aff, pre = T("te1"), T("aff"), T("pre")
            nc.vector.tensor_tensor(te1[:], g1[:], pt3[:, :, 3], op=MUL)
            nc.vector.tensor_tensor(aff[:], coef[:], pt3[:, :, 4], op=MUL)
            nc.vector.tensor_tensor(aff[:], te1[:], aff[:], op=ADD)
            nc.vector.tensor_tensor(aff[:], aff[:], pslice(i, 2), op=ADD)
            nc.vector.tensor_tensor(pre[:], aff[:], pslice(i, 3), op=MUL)
            outv = T("outv")
            if i < NB - 1:
                sil = T("sil")
                nc.scalar.activation(sil[:], pre[:], ACTF.Silu)
                nc.vector.tensor_tensor(outv[:], sil[:], pslice(i, 4), op=MUL)
            else:
                nc.vector.tensor_tensor(outv[:], pre[:], pslice(i, 4), op=MUL)

            # ============ scatter / all-gather ============
            # payload: cc_in[p*RB + rb] = outv[p, rb] (contiguous 16B per
            # partition); gathered payload idx = k*512 + p*4 + rb, which the
            # host maps back to rows via the c-axis permutation
            if i < NB - 1:
                cc_in = dramp.tile([RPC], F32, tag="cci", name="cci")
                cc_out = dramp.tile([W], F32, tag="cco", name="cco")
                nc.sync.dma_start(
                    out=cc_in[:].rearrange("(p rb) -> p rb", rb=RB), in_=outv[:])
                nc.gpsimd.collective_compute(
                    "AllGather",
                    mybir.AluOpType.bypass,
                    replica_groups=[list(range(NC))],
                    ins=[cc_in[:].opt()],
                    outs=[cc_out[:].opt()],
                )
                prev_cc_out = cc_out
                # keep-warm ladder: gpsimd ops pace the boundary window; a
                # 4-col matmul after each keeps PE activity inside every
                # HAM MID window so the next sweep starts at 2.4 GHz
                ps_w = psump.tile([128, 8], F32, tag="ps_w", name="ps_w")
                for k in range(NWARM):
                    col = warm_sb[:, k * 256:(k + 1) * 256]
                    nc.gpsimd.tensor_scalar(col, col, 0.0, None, op0=ADD)
                    nc.tensor.matmul(
                        ps_w[0:1, 0:4], lhsT=col[:, 0:1], rhs=col[:, 0:4],
                        start=True, stop=True)
            else:
                nc.sync.dma_start(
                    out=y_d.ap().rearrange("(p rb) -> p rb", rb=RB), in_=outv[:])


def _build_program():
    nc = bacc.Bacc("TRN2", target_bir_lowering=False, debug=False,
                   num_devices=NC)
    wm_d = nc.dram_tensor("wm", [NB, 128, CB, RPC], BF16, kind="ExternalInput")
    xf_d = nc.dram_tensor("xf", [128, CB], F32, kind="ExternalInput")
    pf_d = nc.dram_tensor("pf", [128, NB * 6 * RB], F32, kind="ExternalInput")
    sel_d = nc.dram_tensor("sel", [35, 8], F32, kind="ExternalInput")
    y_d = nc.dram_tensor("y", [RPC], F32, kind="ExternalOutput")
    with tile.TileContext(nc) as tc:
        _kernel_body(nc, tc, wm_d, xf_d, pf_d, sel_d, y_d)
    nc.compile()
    return nc


# payload idx -> neuron row: idx = k*512 + p*4 + rb  <->  r = k*512 + rb*128 + p
def _cperm():
    idx = np.arange(W)
    return (idx >> 9) * 512 + (idx & 3) * 128 + ((idx >> 2) & 127)


def _pack_inputs(x, weights, masks, biases, gamma, beta, gain, amplification):
    bf = ml_dtypes.bfloat16
    w32 = np.asarray(weights, np.float32)
    m32 = np.asarray(masks, np.float32)
    wm = (w32 * m32).astype(bf)
    # guarantee (wm != 0) == mask: clamp impossible underflow collisions
    bad = (m32 != 0) & (wm == 0)
    if bad.any():
        wm[bad] = np.float32(2.0 ** -120)

    cperm = _cperm()
    # [NB, W(r), W(c)] -> cols permuted to payload order -> [p, j, k, rr]
    # with r = k*RPC + rr (rr = rb*128 + p_out), c = cperm[p*CB + j]
    wmf = wm[:, :, cperm].reshape(NB, NC, RPC, 128, CB).transpose(0, 3, 4, 1, 2)

    x32 = np.asarray(x, np.float32)
    xf = np.ascontiguousarray(x32[cperm].reshape(128, CB))

    # params: [NB*W] -> [NB, NC, RB, 128] (row r = k*RPC + rb*128 + p)
    def fold_param(a):
        return np.asarray(a, np.float32).reshape(NB, NC, RB, 128)

    n_in = m32.sum(axis=2).reshape(-1)           # [NB*W]
    rn = (1.0 / n_in).astype(np.float32)
    ps = [fold_param(a) for a in (gamma, beta, biases, gain, amplification, rn)]
    pall = np.stack(ps, axis=1)  # [NB, 6, NC, RB, 128]

    # fold-transpose selector: rows = sb partitions, cols = epilogue slots
    # sb rows [0..4] = [s1hi, s1lo, n_in, sqhi, sqlo], [32..34] = [t1hi, t1lo, rowWM]
    sel = np.zeros((35, 8), np.float32)
    sel[0, 0] = sel[1, 0] = 1.0   # s1 = s1hi + s1lo
    sel[2, 1] = 1.0               # n_in (unused)
    sel[3, 2] = sel[4, 2] = 1.0   # s2 = sqhi + sqlo
    sel[32, 3] = sel[33, 3] = 1.0  # t1 = t1hi + t1lo
    sel[34, 4] = 1.0              # rowWM

    in_maps = []
    for k in range(NC):
        pf = np.ascontiguousarray(
            pall[:, :, k].transpose(3, 0, 1, 2).reshape(128, NB * 6 * RB))
        in_maps.append({
            "wm": np.ascontiguousarray(wmf[:, :, :, k, :]),
            "xf": xf,
            "pf": pf,
            "sel": sel,
        })
    return in_maps


def _get_program():
    global _CACHED
    if _CACHED is None:
        _CACHED = _build_program()
    return _CACHED


def _run(in_maps, **kw):
    nc = _get_program()
    return bass_utils.run_bass_kernel_spmd(
        nc, in_maps, core_ids=list(range(NC)), **kw)


def _unfold_y(shard):
    # y shard idx p*RB + rb = local row rb*128 + p
    return np.ascontiguousarray(
        np.asarray(shard, np.float32).reshape(128, RB).T.reshape(-1))


def kernel(x, weights, masks, biases, gamma, beta, gain, amplification):
    in_maps = _pack_inputs(x, weights, masks, biases, gamma, beta, gain,
                           amplification)
    res = _run(in_maps)
    return np.concatenate([_unfold_y(res.results[k]["y"]) for k in range(NC)])


def run_traced(inputs, **kw):
    """For test.py: same as kernel() but with NTFF profiling enabled."""
    in_maps = _pack_inputs(**inputs)
    res = _run(in_maps, trace=True, **kw)
    y = np.concatenate([_unfold_y(res.results[k]["y"]) for k in range(NC)])
    return y, res


# revision 13
# speedup vs baseline: 1.3363x; 1.3363x over previous
"""Trainium2 Bass kernel: topo-batched masked-norm NN forward (gnn_message_passing).

Math per topo batch i (reference.py):
    vals = previous layer activations [W]
    n_in[r]  = sum_c M[r,c]                       (host-precomputed -> rn = 1/n_in)
    mean[r]  = (M @ vals)[r] / n_in[r]
    var[r]   = (M @ vals^2)[r] / n_in[r] - mean[r]^2
    rs[r]    = 1/sqrt(var[r] + EPS)
    affine[r]= gamma*rs*(WM @ vals)[r] + (beta - gamma*rs*mean)[r]*(WM @ 1)[r] + bias[r]
        where WM = W (.) M   (algebraic expansion of the masked-norm + masked affine)
    out = silu(affine*gain)*amp   (last batch: identity instead of silu)

Distribution: rows (output neurons) sharded across 8 cores (512 rows/core);
the 4096-vector of activations is all-gathered between batches.

v2 design (from trace analysis of the hi/lo baseline at 529us):
  - ONLY WM is shipped, as single bf16 (weights were f32-split before; the
    2e-2 tolerance leaves ample room). The 0/1 mask is derived ON DEVICE as
    M = (WM != 0) -- one DVE pass -- since WM is nonzero exactly where the
    mask is set (host clamps the impossible collisions). HBM traffic drops
    2.5x and the 16 per-batch DVE mask-multiplies disappear.
  - The matvec sweep runs stats (M@*, 5 stationary cols) and affine (WM@*,
    3 cols) CONCURRENTLY in different PE column groups via tile_position
    (0,0)/(0,32): the two 512-col streams overlap, halving sweep time.
  - Bulk weight DMA rides the scalar (ACT) HWDGE ring; the collective
    payload + vals DMAs ride the sync (SP) ring so the AllGather trigger
    no longer queues behind megabyte weight transfers.
  - The all-gather payload is consumed through a host-side column
    permutation so each partition reads one contiguous 128B run instead of
    eight 16B strides.
  - Epilogue: 1/n_in from the host, rsqrt = DVE reciprocal + ACT Sqrt,
    silu on ACT -- ~15 short DVE ops instead of ~25.
"""

import numpy as np
import ml_dtypes

import concourse.bass as bass
import concourse.bacc as bacc
import concourse.tile as tile
import concourse.mybir as mybir
from concourse import bass_utils

L, W, NC = 8, 4096, 8
NB = L - 1                # 7 topo batches
RPC = W // NC             # 512 rows per core
CB = W // 128             # 32 contraction blocks of 128
RB = RPC // 128           # 4 row blocks of 128 per core
HJ = CB // 2              # c-blocks per half tile (16)
EPS = 1e-5

BF16 = mybir.dt.bfloat16
F32 = mybir.dt.float32
I32 = mybir.dt.int32
ADD = mybir.AluOpType.add
SUB = mybir.AluOpType.subtract
MUL = mybir.AluOpType.mult
NE = mybir.AluOpType.not_equal
RSHIFT = mybir.AluOpType.logical_shift_right
ACTF = mybir.ActivationFunctionType

_CACHED = None


def _kernel_body(nc, tc, wm_d, xf_d, pf_d, sel_d, y_d):
    NP = 6  # per-row params: gamma, beta, bias, gain, amp, rn
    NQ = 4                    # weight DMA chunks per batch (8 c-blocks each)
    QJ = CB // NQ
    with (
        tc.tile_pool(name="const", bufs=1) as constp,
        tc.tile_pool(name="wm", bufs=2 * NQ) as wmp,
        tc.tile_pool(name="mk", bufs=2 * NQ) as mkp,
        tc.tile_pool(name="vals", bufs=2) as valsp,
        tc.tile_pool(name="ep", bufs=2) as epp,
        tc.tile_pool(name="sb8", bufs=1) as sb8p,
        tc.tile_pool(name="psum", bufs=1, space="PSUM") as psump,
        tc.tile_pool(name="dram", bufs=2, space="DRAM") as dramp,
    ):
        # ---- persistent: per-row params, folded [128, NB*NP*RB] ----
        params = constp.tile([128, NB * NP * RB], F32)
        nc.sync.dma_start(out=params[:], in_=pf_d.ap())

        def pslice(i, s):
            o = (i * NP + s) * RB
            return params[:, o:o + RB]

        # combined row-selector for the PSUM fold transpose (rows 0-4 stats,
        # rows 32-34 affine; rows 5-31 are zero)
        sel = constp.tile([128, 8], F32, name="sel")
        nc.sync.dma_start(out=sel[0:35, :], in_=sel_d.ap())

        # ---- persistent: per-batch stationary vectors [128, CB*5] bf16 ----
        # col layout per c-block j: [vhi, vlo, ones, sqhi, sqlo]
        vstat = constp.tile([128, CB * 5], BF16)
        v3 = vstat[:].rearrange("p (j s) -> p j s", s=5)
        nc.vector.memset(v3[:, :, 2], 1.0)

        # single SBUF staging tile for the fold transpose; partitions 5-31
        # feed zero selector rows but are still streamed through the PE, so
        # they must hold real numbers, not stale-SBUF bit patterns
        sb = sb8p.tile([128, 512], F32, tag="sb", name="sb")
        nc.vector.memset(sb[0:32, :], 0.0)

        # ---- weight streaming + on-device mask (prefetched one batch ahead:
        # the DMA lands and the IS_NE mask-derive runs during the PREVIOUS
        # batch's sweep, keeping both off the epilogue's critical path).
        # The dma_starts are issued at high priority: at natural (body-order)
        # priority the list scheduler plans the dispatch after the previous
        # epilogue, which at runtime starves the sweep-window of weights ----
        def issue_weights(i):
            wm_t, m_t = [], []
            for q in range(NQ):
                wt = wmp.tile([128, QJ * RPC], BF16, tag="wm", name="wm")
                with tc.high_priority(offset=250):
                    nc.scalar.dma_start(
                        out=wt[:].rearrange("p (a b) -> p a b", b=RPC),
                        in_=wm_d[i][:, q * QJ:(q + 1) * QJ, :],
                    )
                mt = mkp.tile([128, QJ * RPC], BF16, tag="mk", name="mk")
                EW = QJ * RPC // 2
                for e in range(2):
                    nc.vector.tensor_scalar(
                        mt[:, e * EW:(e + 1) * EW],
                        wt[:, e * EW:(e + 1) * EW],
                        0.0, None, op0=NE)
                wm_t.append(wt)
                m_t.append(mt)
            return wm_t, m_t

        cur_w = issue_weights(0)
        prev_cc_out = None
        for i in range(NB):
            wm_t, m_t = cur_w

            # ============ vals -> vstat ============
            vals = valsp.tile([128, CB], F32, tag="vals", name="vals")
            if i == 0:
                nc.sync.dma_start(out=vals[:], in_=xf_d.ap())
            else:
                # payload idx = p*CB + j by construction (host permutes the
                # weight c-axis to match) => contiguous 128B per partition
                nc.sync.dma_start(
                    out=vals[:],
                    in_=prev_cc_out.rearrange("(p j) -> p j", j=CB),
                )
            tmp_a = epp.tile([128, CB], F32, tag="vtmp_a", name="vtmp_a")
            tmp_sq = epp.tile([128, CB], F32, tag="vtmp_sq", name="vtmp_sq")
            nc.vector.tensor_copy(v3[:, :, 0], vals[:])            # vhi
            nc.vector.tensor_copy(tmp_a[:], v3[:, :, 0])
            nc.vector.tensor_tensor(v3[:, :, 1], vals[:], tmp_a[:], op=SUB)
            nc.vector.tensor_tensor(tmp_sq[:], vals[:], vals[:], op=MUL)
            nc.vector.tensor_copy(v3[:, :, 3], tmp_sq[:])          # sqhi
            nc.vector.tensor_copy(tmp_a[:], v3[:, :, 3])
            nc.vector.tensor_tensor(v3[:, :, 4], tmp_sq[:], tmp_a[:], op=SUB)

            # prefetch next batch's weights + mask (runs under this sweep)
            if i + 1 < NB:
                cur_w = issue_weights(i + 1)

            # ============ matvec sweep (two concurrent column groups) ======
            # ps_st rows 0-4: [M@vhi, M@vlo, M@1, M@sqhi, M@sqlo]
            # ps_af rows 32-34: [WM@vhi, WM@vlo, WM@1]
            ps_st = psump.tile([128, 512], F32, tag="ps_st", name="ps_st")
            ps_af = psump.tile([128, 512], F32, tag="ps_af", name="ps_af")
            for j in range(CB):
                q, jq = divmod(j, QJ)
                rhs_m = m_t[q][:, jq * RPC:(jq + 1) * RPC]
                rhs_w = wm_t[q][:, jq * RPC:(jq + 1) * RPC]
                st, sp = (j == 0), (j == CB - 1)
                nc.tensor.matmul(ps_st[0:5, :], lhsT=vstat[:, j * 5:j * 5 + 5],
                                 rhs=rhs_m, start=st, stop=sp,
                                 tile_position=(0, 0))
                nc.tensor.matmul(ps_af[32:35, :], lhsT=vstat[:, j * 5:j * 5 + 3],
                                 rhs=rhs_w, start=st, stop=sp,
                                 tile_position=(0, 32))

            # ============ transpose to fold layout ============
            # sb rows: 0-4 = stats, 32-34 = affine; one 35-row selector
            # matmul per row block lands [128, 8] in PSUM with cols
            # [s1, n_in, s2, t1, rowWM] (bf16 hi/lo partials summed free)
            nc.vector.tensor_copy(sb[0:5, :], ps_st[0:5, :])
            nc.vector.tensor_copy(sb[32:35, :], ps_af[32:35, :])
            ps_t = psump.tile([128, RB * 512], F32, tag="ps_t", name="ps_t")
            for rb in range(RB):
                nc.tensor.matmul(
                    ps_t[:, rb * 512:rb * 512 + 8],
                    lhsT=sb[0:35, rb * 128:(rb + 1) * 128],
                    rhs=sel[0:35, :], start=True, stop=True)
            pt3 = ps_t[:].rearrange("p (rb s) -> p rb s", s=512)

            # ============ epilogue (all [128, RB] f32) ============
            def T(tag):
                return epp.tile([128, RB], F32, tag=tag, name=tag)

            # pt3 cols: 0=s1, 1=n_in(unused), 2=s2, 3=t1, 4=rowWM
            # params s: 0=gamma 1=beta 2=bias 3=gain 4=amp 5=rn
            mean, ex2, msq, vpe = T("mean"), T("ex2"), T("msq"), T("vpe")
            nc.vector.tensor_tensor(mean[:], pt3[:, :, 0], pslice(i, 5), op=MUL)
            nc.vector.tensor_tensor(ex2[:], pt3[:, :, 2], pslice(i, 5), op=MUL)
            nc.vector.tensor_tensor(msq[:], mean[:], mean[:], op=MUL)
            nc.vector.scalar_tensor_tensor(
                vpe[:], msq[:], -1.0, ex2[:], op0=MUL, op1=ADD)
            nc.vector.tensor_scalar(vpe[:], vpe[:], EPS, None, op0=ADD)
            # rs = 1/sqrt(vpe): Quake seed + 2 Newton iterations (DVE only --
            # an ACT Sqrt would thrash the activation table against Silu)
            rs = T("rs")
            nc.vector.tensor_scalar(
                rs[:].bitcast(I32), vpe[:].bitcast(I32), 1, None, op0=RSHIFT)
            nc.vector.tensor_scalar(
                rs[:].bitcast(I32), rs[:].bitcast(I32), -1, 0x5F3759DF,
                op0=MUL, op1=ADD)
            nra, nrb = T("nra"), T("nrb")
            for _ in range(2):
                nc.vector.tensor_tensor(nra[:], rs[:], rs[:], op=MUL)
                nc.vector.tensor_tensor(nrb[:], nra[:], vpe[:], op=MUL)
                nc.vector.tensor_scalar(nrb[:], nrb[:], -0.5, 1.5, op0=MUL, op1=ADD)
                nc.vector.tensor_tensor(rs[:], rs[:], nrb[:], op=MUL)
            g1, gm, coef = T("g1"), T("gm"), T("coef")
            nc.vector.tensor_tensor(g1[:], pslice(i, 0), rs[:], op=MUL)
            nc.vector.tensor_tensor(gm[:], g1[:], mean[:], op=MUL)
            nc.vector.tensor_tensor(coef[:], pslice(i, 1), gm[:], op=SUB)
            te1, aff, pre = T("te1"), T("aff"), T("pre")
            nc.vector.tensor_tensor(te1[:], g1[:], pt3[:, :, 3], op=MUL)
            nc.vector.tensor_tensor(aff[:], coef[:], pt3[:, :, 4], op=MUL)
            nc.vector.tensor_tensor(aff[:], te1[:], aff[:], op=ADD)
            nc.vector.tensor_tensor(aff[:], aff[:], pslice(i, 2), op=ADD)
            nc.vector.tensor_tensor(pre[:], aff[:], pslice(i, 3), op=MUL)
            outv = T("outv")
            if i < NB - 1:
                sil = T("sil")
                nc.scalar.activation(sil[:], pre[:], ACTF.Silu)
                nc.vector.tensor_tensor(outv[:], sil[:], pslice(i, 4), op=MUL)
            else:
                nc.vector.tensor_tensor(outv[:], pre[:], pslice(i, 4), op=MUL)

            # ============ scatter / all-gather ============
            # payload: cc_in[p*RB + rb] = outv[p, rb] (contiguous 16B per
            # partition); gathered payload idx = k*512 + p*4 + rb, which the
            # host maps back to rows via the c-axis permutation
            if i < NB - 1:
                cc_in = dramp.tile([RPC], F32, tag="cci", name="cci")
                cc_out = dramp.tile([W], F32, tag="cco", name="cco")
                nc.sync.dma_start(
                    out=cc_in[:].rearrange("(p rb) -> p rb", rb=RB), in_=outv[:])
                nc.gpsimd.collective_compute(
                    "AllGather",
                    mybir.AluOpType.bypass,
                    replica_groups=[list(range(NC))],
                    ins=[cc_in[:].opt()],
                    outs=[cc_out[:].opt()],
                )
                prev_cc_out = cc_out
            else:
                nc.sync.dma_start(
                    out=y_d.ap().rearrange("(p rb) -> p rb", rb=RB), in_=outv[:])


def _build_program():
    nc = bacc.Bacc("TRN2", target_bir_lowering=False, debug=False,
                   num_devices=NC)
    wm_d = nc.dram_tensor("wm", [NB, 128, CB, RPC], BF16, kind="ExternalInput")
    xf_d = nc.dram_tensor("xf", [128, CB], F32, kind="ExternalInput")
    pf_d = nc.dram_tensor("pf", [128, NB * 6 * RB], F32, kind="ExternalInput")
    sel_d = nc.dram_tensor("sel", [35, 8], F32, kind="ExternalInput")
    y_d = nc.dram_tensor("y", [RPC], F32, kind="ExternalOutput")
    with tile.TileContext(nc) as tc:
        _kernel_body(nc, tc, wm_d, xf_d, pf_d, sel_d, y_d)
    nc.compile()
    return nc


# payload idx -> neuron row: idx = k*512 + p*4 + rb  <->  r = k*512 + rb*128 + p
def _cperm():
    idx = np.arange(W)
    return (idx >> 9) * 512 + (idx & 3) * 128 + ((idx >> 2) & 127)


def _pack_inputs(x, weights, masks, biases, gamma, beta, gain, amplification):
    bf = ml_dtypes.bfloat16
    w32 = np.asarray(weights, np.float32)
    m32 = np.asarray(masks, np.float32)
    wm = (w32 * m32).astype(bf)
    # guarantee (wm != 0) == mask: clamp impossible underflow collisions
    bad = (m32 != 0) & (wm == 0)
    if bad.any():
        wm[bad] = np.float32(2.0 ** -120)

    cperm = _cperm()
    # [NB, W(r), W(c)] -> cols permuted to payload order -> [p, j, k, rr]
    # with r = k*RPC + rr (rr = rb*128 + p_out), c = cperm[p*CB + j]
    wmf = wm[:, :, cperm].reshape(NB, NC, RPC, 128, CB).transpose(0, 3, 4, 1, 2)

    x32 = np.asarray(x, np.float32)
    xf = np.ascontiguousarray(x32[cperm].reshape(128, CB))

    # params: [NB*W] -> [NB, NC, RB, 128] (row r = k*RPC + rb*128 + p)
    def fold_param(a):
        return np.asarray(a, np.float32).reshape(NB, NC, RB, 128)

    n_in = m32.sum(axis=2).reshape(-1)           # [NB*W]
    rn = (1.0 / n_in).astype(np.float32)
    ps = [fold_param(a) for a in (gamma, beta, biases, gain, amplification, rn)]
    pall = np.stack(ps, axis=1)  # [NB, 6, NC, RB, 128]

    # fold-transpose selector: rows = sb partitions, cols = epilogue slots
    # sb rows [0..4] = [s1hi, s1lo, n_in, sqhi, sqlo], [32..34] = [t1hi, t1lo, rowWM]
    sel = np.zeros((35, 8), np.float32)
    sel[0, 0] = sel[1, 0] = 1.0   # s1 = s1hi + s1lo
    sel[2, 1] = 1.0               # n_in (unused)
    sel[3, 2] = sel[4, 2] = 1.0   # s2 = sqhi + sqlo
    sel[32, 3] = sel[33, 3] = 1.0  # t1 = t1hi + t1lo
    sel[34, 4] = 1.0              # rowWM

    in_maps = []
    for k in range(NC):
        pf = np.ascontiguousarray(
            pall[:, :, k].transpose(3, 0, 1, 2).reshape(128, NB * 6 * RB))
        in_maps.append({
            "wm": np.ascontiguousarray(wmf[:, :, :, k, :]),
            "xf": xf,
            "pf": pf,
            "sel": sel,
        })
    return in_maps


def _get_program():
    global _CACHED
    if _CACHED is None:
        _CACHED = _build_program()
    return _CACHED


def _run(in_maps, **kw):
    nc = _get_program()
    return bass_utils.run_bass_kernel_spmd(
        nc, in_maps, core_ids=list(range(NC)), **kw)


def _unfold_y(shard):
    # y shard idx p*RB + rb = local row rb*128 + p
    return np.ascontiguousarray(
        np.asarray(shard, np.float32).reshape(128, RB).T.reshape(-1))


def kernel(x, weights, masks, biases, gamma, beta, gain, amplification):
    in_maps = _pack_inputs(x, weights, masks, biases, gamma, beta, gain,
                           amplification)
    res = _run(in_maps)
    return np.concatenate([_unfold_y(res.results[k]["y"]) for k in range(NC)])


def run_traced(inputs, **kw):
    """For test.py: same as kernel() but with NTFF profiling enabled."""
    in_maps = _pack_inputs(**inputs)
    res = _run(in_maps, trace=True, **kw)
    y = np.concatenate([_unfold_y(res.results[k]["y"]) for k in range(NC)])
    return y, res
